# Initial kernel scaffold
#
"""Trainium2 Bass kernel for a dense transformer block (B=4096, T=32, C=180,
H=6 heads, head_dim=30): LN -> causal MHA -> residual -> LN -> MLP(180->720
->180, relu) -> residual.

Data-parallel over 8 NeuronCores: batch split 512 sequences/core, weights
replicated. Per 512-token tile:
  - LN1 with rstd = exp(-0.5*ln(var+eps)) so all scalar-engine activations
    stay inside one hw function table set (no ACT_TABLE_LOAD churn).
  - QK projections in channel-major [ch, tok]; scores computed TRANSPOSED
    (stationary=k block, moving=q block) so softmax weights come out in
    [s, t] layout and feed the AV matmul directly as the stationary
    operand -- no PE transpose / PSUM round trip of the attention matrix.
  - V computed token-major [tok, d] (stationary=h^T chunk, moving=wv), with
    a ones-column injected per head so the AV matmul also produces the
    softmax denominator for free.
  - exp() writes bf16 straight from PSUM; causal mask applied in bf16 SBUF.
  - MLP as in the standard channel-major pipeline, residual adds on DVE,
    the attention residual add on the (otherwise idle) GpSimd engine.

Self-contained: only needs numpy + the concourse (bass) package the runtime
environment provides on sys.path.
"""

import ml_dtypes
import numpy as np

import concourse.bass as bass
import concourse.tile as tile
from concourse import mybir
from concourse.bass_utils import run_bass_kernel_spmd

B, T, C, H, HD = 4096, 32, 180, 6, 30
DP = 32                   # padded head dim
D4 = 4 * C                # 720
EPS = 1e-5
NCORES = 8
F32 = mybir.dt.float32
MMDT = mybir.dt.bfloat16  # matmul operand dtype (PSUM accumulation stays fp32)

KA, KB = 128, C - 128     # contraction split of C=180
NT = 512                  # tokens per tile (16 sequences)
NCH = NT // 128           # 4 chunks of 128 tokens per tile
QKW = 2 * H * DP          # 384 columns of fused q|k weight
STAGE = 11                # debug: truncate tile body after stage N (11=full)


def _split_multi_waits(nc):
    """This walrus build accepts at most one sync wait per instruction; hoist
    extra waits emitted by Tile onto NoOps inserted just before, same engine."""
    ctr = 0
    for fn in nc.m.functions:
        for bb in fn.blocks:
            lst = bb.instructions
            i = 0
            while i < len(lst):
                inst = lst[i]
                si = inst.sync_info
                waits = list(si.on_wait) if si and si.on_wait else []
                if len(waits) > 1:
                    nops = []
                    for w in waits[:-1]:
                        ctr += 1
                        nop = mybir.InstNoOp(
                            name=f"T-wsplit-{ctr}",
                            opcode="NoOp",
                            engine=inst.engine,
                            ins=[],
                            outs=[],
                        )
                        nop.sync_info = mybir.SyncInfo(on_wait=[w], on_update=[])
                        nops.append(nop)
                    si.on_wait = waits[-1:]
                    inst.sync_info = si
                    for j, nop in enumerate(nops):
                        lst.insert(i + j, nop)
                    i += len(nops)
                i += 1


def _bc(ap, new_ap, extra_offset=0):
    """AP with a hand-built access-pattern list (stride-0 broadcasts etc.)."""
    return bass.AP(tensor=ap.tensor, offset=ap.offset + extra_offset, ap=new_ap)


def build(n_tok, has_bqkv, has_b1, has_b2, legalize=True):
    nc = bass.Bass()
    ts, ds = bass.ts, bass.ds
    AF = mybir.ActivationFunctionType
    ALU = mybir.AluOpType
    ntiles = n_tok // NT

    x_d = nc.declare_dram_parameter("x", [n_tok, C], F32, isOutput=False)
    wqk_d = nc.declare_dram_parameter("wqk", [C, QKW], MMDT, isOutput=False)
    wv_d = nc.declare_dram_parameter("wv", [C, H * DP], MMDT, isOutput=False)
    bqkv_d = nc.declare_dram_parameter("bqkv", [QKW, 1], F32, isOutput=False)
    bvrep_d = nc.declare_dram_parameter("bvrep", [128, H * DP], F32, isOutput=False)
    w1_d = nc.declare_dram_parameter("w1", [C, D4], MMDT, isOutput=False)
    b1_d = nc.declare_dram_parameter("b1", [D4, 1], F32, isOutput=False)
    w2t_d = nc.declare_dram_parameter("w2t", [120, 6, C], MMDT, isOutput=False)
    b2_d = nc.declare_dram_parameter("b2", [C, 1], F32, isOutput=False)
    mask_d = nc.declare_dram_parameter("maskT", [128, T], MMDT, isOutput=False)
    id_d = nc.declare_dram_parameter("ident", [128, 128], F32, isOutput=False)
    out_d = nc.declare_dram_parameter("out", [n_tok, C], F32, isOutput=True)

    with tile.TileContext(nc) as tc:
        with (
            tc.tile_pool(name="const", bufs=1) as const,
            tc.tile_pool(name="tok", bufs=4) as tok,
            tc.tile_pool(name="ln", bufs=2) as lnp,
            tc.tile_pool(name="tr", bufs=2) as tr,
            tc.tile_pool(name="qk", bufs=2) as qkp,
            tc.tile_pool(name="at", bufs=2) as at,
            tc.tile_pool(name="mlp", bufs=2) as mlpp,
            tc.tile_pool(name="ps", bufs=1, space="PSUM") as ps,
        ):
            # ---- constants / weights ----
            ident = const.tile([128, 128], F32)
            nc.sync.dma_start(out=ident, in_=id_d[:, :])
            maskT = const.tile([128, T], MMDT)
            nc.sync.dma_start(out=maskT, in_=mask_d[:, :])

            wqkA = const.tile([KA, QKW], MMDT)
            nc.sync.dma_start(out=wqkA, in_=wqk_d[0:KA, :])
            wqkB = const.tile([KB, QKW], MMDT)
            nc.sync.dma_start(out=wqkB, in_=wqk_d[KA:C, :])
            wvA = const.tile([KA, H * DP], MMDT)
            nc.gpsimd.dma_start(out=wvA, in_=wv_d[0:KA, :])
            wvB = const.tile([KB, H * DP], MMDT)
            nc.gpsimd.dma_start(out=wvB, in_=wv_d[KA:C, :])
            w1A = const.tile([KA, D4], MMDT)
            nc.gpsimd.dma_start(out=w1A, in_=w1_d[0:KA, :])
            w1B = const.tile([KB, D4], MMDT)
            nc.gpsimd.dma_start(out=w1B, in_=w1_d[KA:C, :])
            w2t = const.tile([120, 6, C], MMDT)
            nc.gpsimd.dma_start(out=w2t, in_=w2t_d[:, :, :])

            # q|k band biases: (offset, rows) for qA,qB,kA,kB
            BANDS = [(0, 128), (128, 64), (192, 128), (320, 64)]
            bqk = []
            for bi, (c0, rows) in enumerate(BANDS):
                t_ = const.tile([rows, 1], F32, tag=f"bqk{bi}")
                if has_bqkv:
                    nc.sync.dma_start(out=t_, in_=bqkv_d[c0 : c0 + rows, :])
                bqk.append(t_)
            bvrep = const.tile([128, H * DP], F32)
            if has_bqkv:
                nc.sync.dma_start(out=bvrep, in_=bvrep_d[:, :])
            b1t = const.tile([120, 6, 1], F32)
            if has_b1:
                nc.sync.dma_start(
                    out=b1t, in_=b1_d.rearrange("(g p) one -> p g one", g=6)
                )
            b2t = const.tile([90, 2, 1], F32)
            if has_b2:
                nc.sync.dma_start(
                    out=b2t, in_=b2_d.rearrange("(m p) one -> p m one", m=2)
                )
            epst = const.tile([128, 1], F32)
            nc.vector.memset(epst, EPS)
            ones1 = const.tile([128, 1], F32)
            nc.vector.memset(ones1, 1.0)
            identb = const.tile([128, 128], MMDT)
            nc.vector.tensor_copy(identb, ident)

            def ln_t(src, sfx):
                """LN of [128, NCH, C] (rstd via exp(-ln/2)) fused with the
                per-chunk PE transposes: each chunk's transpose is emitted
                right after its affine apply (applies split DVE/Act) so the
                PE starts as early as possible. Returns hTA [128,512] +
                hTB [52,512] (bf16, channel-major)."""
                mv = lnp.tile([128, NCH, nc.vector.BN_AGGR_DIM], F32, tag=f"mv{sfx}")
                for j in range(NCH):
                    st = lnp.tile([128, nc.vector.BN_STATS_DIM], F32, tag=f"st{sfx}")
                    nc.vector.bn_stats(out=st, in_=src[:, j, :])
                    nc.vector.bn_aggr(out=mv[:, j, :], in_=st)
                lnv = lnp.tile([128, NCH], F32, tag=f"lnv{sfx}")
                nc.scalar.activation(
                    out=lnv, in_=mv[:, :, 1], func=AF.Ln, bias=epst, scale=1.0
                )
                rstd = lnp.tile([128, NCH], F32, tag=f"rstd{sfx}")
                nc.scalar.activation(out=rstd, in_=lnv, func=AF.Exp, scale=-0.5)
                nmr = lnp.tile([128, NCH], F32, tag=f"nmr{sfx}")
                nc.vector.scalar_tensor_tensor(
                    out=nmr, in0=mv[:, :, 0], scalar=-1.0, in1=rstd,
                    op0=ALU.mult, op1=ALU.mult,
                )
                xh = lnp.tile([128, NCH, C], MMDT, tag=f"xh{sfx}")
                pT = ps.tile(
                    [128, 1024], MMDT, tag="P4" if sfx == "1" else "P0",
                    name=f"pT{sfx}",
                )
                for j in range(NCH):
                    if j % 2 == 0:
                        nc.vector.scalar_tensor_tensor(
                            out=xh[:, j, :],
                            in0=src[:, j, :],
                            scalar=rstd[:, j : j + 1],
                            in1=_bc(nmr, [nmr.ap[0], [0, C]], j),
                            op0=ALU.mult,
                            op1=ALU.add,
                        )
                    else:
                        nc.scalar.activation(
                            out=xh[:, j, :],
                            in_=src[:, j, :],
                            func=AF.Identity,
                            bias=nmr[:, j : j + 1],
                            scale=rstd[:, j : j + 1],
                        )
                    nc.tensor.transpose(pT[:, ts(j, 128)], xh[:, j, 0:128], identb)
                    nc.tensor.transpose(
                        pT[0:KB, 512 + 128 * j : 512 + 128 * (j + 1)],
                        xh[:, j, 128:C],
                        identb,
                    )
                hTA = tr.tile([KA, 512], MMDT, tag=f"hTA{sfx}")
                nc.vector.tensor_copy(hTA, pT[:, 0:512])
                hTB = tr.tile([KB, 512], MMDT, tag=f"hTB{sfx}")
                nc.scalar.copy(hTB, pT[0:KB, 512:1024])
                return hTA, hTB

            def stage_front1(it):
                """LN1 -> T1 -> QK -> V -> scores for tile it."""
                st = {}
                tok0 = it * NT
                x_t = tok.tile([128, NCH, C], F32, tag="x_t")
                nc.sync.dma_start(
                    out=x_t,
                    in_=x_d[tok0 : tok0 + NT, :].rearrange("(n p) d -> p n d", p=128),
                )
                st["x_t"] = x_t

                hTA, hTB = ln_t(x_t, "1")

                QKT = ("P5", "P2", "P6", "P0")
                qk_sb = []
                for bi, (c0, rows) in enumerate(BANDS):
                    p_mm = ps.tile([rows, 512], F32, tag=QKT[bi])
                    nc.tensor.matmul(
                        p_mm, wqkA[:, c0 : c0 + rows], hTA, start=True, stop=False
                    )
                    nc.tensor.matmul(
                        p_mm, wqkB[:, c0 : c0 + rows], hTB, start=False, stop=True
                    )
                    dst = qkp.tile([rows, 512], MMDT, tag=f"qk{bi}")
                    eng_dve = bi % 2 == 0
                    if has_bqkv:
                        if eng_dve:
                            nc.vector.tensor_scalar(
                                out=dst, in0=p_mm, scalar1=bqk[bi], scalar2=None,
                                op0=ALU.add,
                            )
                        else:
                            nc.scalar.activation(
                                out=dst, in_=p_mm, func=AF.Identity,
                                bias=bqk[bi], scale=1.0,
                            )
                    elif eng_dve:
                        nc.vector.tensor_copy(dst, p_mm)
                    else:
                        nc.scalar.copy(dst, p_mm)
                    qk_sb.append(dst)
                qA, qB, kA, kB = qk_sb

                vps = [
                    ps.tile([128, 2, H * DP], F32, tag="P7", name="vps0"),
                    ps.tile([128, 2, H * DP], F32, tag="P1", name="vps1"),
                ]
                for j in range(NCH):
                    dst = vps[j // 2][:, j % 2, :]
                    nc.tensor.matmul(
                        dst, hTA[:, ts(j, 128)], wvA, start=True, stop=False
                    )
                    nc.tensor.matmul(
                        dst, hTB[:, ts(j, 128)], wvB, start=False, stop=True
                    )
                vp = qkp.tile([128, NCH, H * DP], MMDT, tag="vp")
                nc.scalar.copy(vp[:, 0:2, :], vps[0])
                nc.scalar.copy(vp[:, 2:4, :], vps[1])
                nc.vector.tensor_copy(
                    _bc(vp, [vp.ap[0], [H * DP, NCH], [DP, H]], 30),
                    _bc(ones1, [ones1.ap[0], [0, NCH], [0, H]]),
                )
                st["vp"] = vp

                SCT = ("P4", "P5", "P6", "P7")
                sc = [
                    ps.tile([128, 256 if rr < 2 else 128], F32,
                            tag=SCT[rr], name=f"sc{rr}")
                    for rr in range(4)
                ]
                for rr in range(4):
                    for h in (rr, rr + 4) if rr < 2 else (rr,):
                        col0 = 128 if h >= 4 else 0
                        qs, ks = (qA, kA) if h < 4 else (qB, kB)
                        pb = 32 * rr
                        for g in range(NCH):
                            for c in range(4):
                                sq = 4 * g + c
                                nc.tensor.matmul(
                                    sc[rr][ds(32 * c, 32), ds(col0 + 32 * g, 32)],
                                    ks[ds(pb, 32), ts(sq, T)],
                                    qs[ds(pb, 32), ts(sq, T)],
                                    start=True,
                                    stop=True,
                                    tile_position=(pb, 32 * c),
                                )
                st["sc"] = sc
                return st

            def stage_exp(st):
                """exp + causal mask per head-row bank -> eT (bf16)."""
                sc = st["sc"]
                eT = at.tile([128, H * NCH, T], MMDT, tag="eT")
                for rr in range(4):
                    if rr < 2:
                        o_ap = _bc(
                            eT,
                            [eT.ap[0], [4 * NCH * T, 2], [T, NCH], [1, T]],
                            rr * NCH * T,
                        )
                        i_ap = _bc(
                            sc[rr], [sc[rr].ap[0], [128, 2], [T, NCH], [1, T]]
                        )
                        m_ap = _bc(
                            maskT, [maskT.ap[0], [0, 2], [0, NCH], [1, T]]
                        )
                    else:
                        o_ap = _bc(
                            eT, [eT.ap[0], [T, NCH], [1, T]], rr * NCH * T
                        )
                        i_ap = _bc(sc[rr], [sc[rr].ap[0], [T, NCH], [1, T]])
                        m_ap = _bc(maskT, [maskT.ap[0], [0, NCH], [1, T]])
                    nc.scalar.activation(out=o_ap, in_=i_ap, func=AF.Exp)
                    nc.vector.tensor_mul(o_ap, o_ap, m_ap)
                st["eT"] = eT
                avs = at.tile([128, NCH, H, HD], F32, tag="avs", name="avs")
                st["avs"] = avs

            def stage_av(st, p):
                """AV pass p (chunks 2p, 2p+1) + softmax scale."""
                eT, vp, avs = st["eT"], st["vp"], st["avs"]
                AVT = ("P4", "P5", "P6", "P7")
                avbp = [
                    ps.tile([128, 2, H * DP], F32, tag=AVT[c], name=f"av{c}_{p}")
                    for c in range(4)
                ]
                for c in range(4):
                    for h in range(H):
                        for gg in range(2):
                            g = 2 * p + gg
                            idx2 = h * NCH + g
                            nc.tensor.matmul(
                                avbp[c][ds(32 * c, 32), gg, ds(DP * h, 32)],
                                eT[ds(32 * c, 32), idx2, :],
                                vp[ds(32 * c, 32), g, ds(DP * h, 32)],
                                start=True,
                                stop=True,
                                tile_position=(32 * c, 32 * c),
                            )
                    av32 = avbp[c][ds(32 * c, 32), :, :]
                    R = at.tile([128, 2, H], F32, tag=f"R{c % 2}")
                    rsl = R[ds(32 * c, 32), :, :]
                    nc.vector.reciprocal(
                        rsl,
                        _bc(av32, [av32.ap[0], [H * DP, 2], [DP, H]], 30),
                    )
                    nc.vector.tensor_mul(
                        avs[ds(32 * c, 32), 2 * p : 2 * p + 2],
                        _bc(av32,
                            [av32.ap[0], [H * DP, 2], [DP, H], [1, HD]]),
                        _bc(rsl, [rsl.ap[0], [H, 2], [1, H], [0, HD]]),
                    )

            def stage_resid_ln2(st):
                """x2 = x + attn; LN2; T2 transposes."""
                x_t, avs = st["x_t"], st["avs"]
                x2 = tok.tile([128, NCH, C], F32, tag="x2")
                nc.vector.tensor_add(
                    _bc(x2, [x2.ap[0], [C, NCH], [HD, H], [1, HD]]),
                    _bc(x_t, [x_t.ap[0], [C, NCH], [HD, H], [1, HD]]),
                    avs,
                )
                if has_bqkv:
                    x2v = _bc(x2, [x2.ap[0], [C, NCH], [HD, H], [1, HD]])
                    nc.vector.tensor_add(
                        x2v,
                        x2v,
                        _bc(bvrep, [bvrep.ap[0], [0, NCH], [DP, H], [1, HD]]),
                    )
                st["x2"] = x2
                st["h2"] = ln_t(x2, "2")

            def stage_mlp1(st, gms):
                h2A, h2B = st["h2"]
                M1ROT = ["P0", "P1", "P2"]
                m1 = st.setdefault("m1", {})
                for gm in gms:
                    p_mm = ps.tile([120, 512], F32, tag=M1ROT[gm % 3])
                    nc.tensor.matmul(
                        p_mm, w1A[:, ds(120 * gm, 120)], h2A, start=True, stop=False
                    )
                    nc.tensor.matmul(
                        p_mm, w1B[:, ds(120 * gm, 120)], h2B, start=False, stop=True
                    )
                    dst = mlpp.tile([120, 512], MMDT, tag=f"m1_{gm}")
                    if gm != 0:
                        nc.scalar.activation(
                            out=dst,
                            in_=p_mm,
                            func=AF.Relu,
                            bias=b1t[:, gm, :] if has_b1 else 0.0,
                            scale=1.0,
                        )
                    elif has_b1:
                        nc.vector.tensor_scalar(
                            out=dst,
                            in0=p_mm,
                            scalar1=b1t[:, gm, :],
                            scalar2=0.0,
                            op0=ALU.add,
                            op1=ALU.max,
                        )
                    else:
                        nc.vector.tensor_scalar(
                            out=dst,
                            in0=p_mm,
                            scalar1=0.0,
                            scalar2=None,
                            op0=ALU.max,
                        )
                    m1[gm] = dst

            def stage_mlp2(st):
                m1 = st["m1"]
                mlpT = []
                for mm in range(2):
                    p_mm = ps.tile([90, 512], F32, tag=("P1", "P2")[mm])
                    for k in range(6):
                        nc.tensor.matmul(
                            p_mm,
                            w2t[:, k, ds(90 * mm, 90)],
                            m1[k],
                            start=(k == 0),
                            stop=(k == 5),
                        )
                    dst = mlpp.tile([90, 512], MMDT, tag=f"mlpT{mm}")
                    if has_b2:
                        if mm == 0:
                            nc.vector.tensor_scalar(
                                out=dst, in0=p_mm, scalar1=b2t[:, mm, :],
                                scalar2=None, op0=ALU.add,
                            )
                        else:
                            nc.scalar.activation(
                                out=dst, in_=p_mm, func=AF.Identity,
                                bias=b2t[:, mm, :], scale=1.0,
                            )
                    elif mm == 0:
                        nc.vector.tensor_copy(dst, p_mm)
                    else:
                        nc.scalar.copy(dst, p_mm)
                    mlpT.append(dst)
                st["mlpT"] = mlpT

            def stage_out(st, it):
                """Transpose back + final residual + store for tile it."""
                tok0 = it * NT
                x2, mlpT = st["x2"], st["mlpT"]
                of = tok.tile([128, NCH, C], F32, tag="of")
                for mm in range(2):
                    po = ps.tile([128, NCH, 90], MMDT, tag="P3", name=f"po{mm}")
                    for j in range(NCH):
                        nc.tensor.transpose(
                            po[:, j, :], mlpT[mm][:, ts(j, 128)],
                            identb[0:90, 0:90],
                        )
                    o_ap = _bc(of, [of.ap[0], [C, NCH], [1, 90]], 90 * mm)
                    i_ap = _bc(x2, [x2.ap[0], [C, NCH], [1, 90]], 90 * mm)
                    nc.vector.tensor_add(o_ap, i_ap, po)
                nc.sync.dma_start(
                    out=out_d[tok0 : tok0 + NT, :].rearrange(
                        "(n p) d -> p n d", p=128
                    ),
                    in_=of,
                )

            # Software-pipelined schedule: tile i's MLP/out (back half) is
            # emitted inside tile i+1's attention stalls, so every engine
            # stream keeps cross-tile work adjacent to the other tiles'
            # latency chains.
            prev = None
            prev_it = -1
            for it in range(ntiles):
                st = stage_front1(it)
                stage_exp(st)
                if prev is not None:
                    stage_mlp1(prev, range(0, 1))
                stage_av(st, 0)
                if prev is not None:
                    stage_mlp1(prev, range(1, 2))
                stage_av(st, 1)
                if prev is not None:
                    stage_mlp1(prev, range(2, 3))
                    stage_mlp2(prev)
                    stage_out(prev, prev_it)
                stage_resid_ln2(st)
                stage_mlp1(st, range(3, 6))
                prev, prev_it = st, it
            stage_mlp1(prev, range(0, 3))
            stage_mlp2(prev)
            stage_out(prev, prev_it)

    if legalize:
        _split_multi_waits(nc)
    return nc


def _prep(wq, wk, wv, g1, be1, g2, be2, w1, b1, w2, b2):
    f = np.float32
    wq, wk, wv = (np.asarray(a, f) for a in (wq, wk, wv))
    g1, be1 = np.asarray(g1, f), np.asarray(be1, f)
    g2, be2 = np.asarray(g2, f), np.asarray(be2, f)
    wqk = np.zeros((C, QKW), f)
    wvm = np.zeros((C, H * DP), f)
    for h in range(H):
        wqk[:, DP * h : DP * h + HD] = wq[h] * f(HD) ** -0.5
        wqk[:, H * DP + DP * h : H * DP + DP * h + HD] = wk[h]
        wvm[:, DP * h : DP * h + HD] = wv[h]
    bqk = (be1 @ wqk).reshape(QKW, 1)
    bv = be1 @ wvm
    bvrep = np.tile(bv[None, :], (128, 1))
    wqk *= g1[:, None]
    wvm *= g1[:, None]
    w1 = np.asarray(w1, f)
    w1e = g2[:, None] * w1
    b1e = (np.asarray(b1, f) + be2 @ w1).reshape(D4, 1)
    w2t = np.asarray(w2, f).reshape(6, 120, C).transpose(1, 0, 2)
    maskT = np.tile(
        (np.arange(T)[:, None] <= np.arange(T)[None, :]).astype(f), (128 // T, 1)
    )
    ident = np.eye(128, dtype=f)
    bf = ml_dtypes.bfloat16
    return {
        "wqk": np.ascontiguousarray(wqk.astype(bf)),
        "wv": np.ascontiguousarray(wvm.astype(bf)),
        "bqkv": np.ascontiguousarray(bqk),
        "bvrep": np.ascontiguousarray(bvrep),
        "w1": np.ascontiguousarray(w1e.astype(bf)),
        "b1": np.ascontiguousarray(b1e),
        "w2t": np.ascontiguousarray(w2t.astype(bf)),
        "b2": np.ascontiguousarray(np.asarray(b2, f).reshape(C, 1)),
        "maskT": np.ascontiguousarray(maskT.astype(bf)),
        "ident": ident,
    }


def kernel(x, wq, wk, wv, g1, be1, g2, be2, w1, b1, w2, b2):
    x = np.asarray(x, np.float32)
    shared = _prep(wq, wk, wv, g1, be1, g2, be2, w1, b1, w2, b2)
    n_tok = (B // NCORES) * T
    nc = build(
        n_tok,
        bool(np.any(shared["bqkv"])) or bool(np.any(shared["bvrep"])),
        bool(np.any(shared["b1"])),
        bool(np.any(shared["b2"])),
    )
    shards = x.reshape(NCORES, n_tok, C)
    in_maps = [
        {"x": np.ascontiguousarray(shards[i]), **shared} for i in range(NCORES)
    ]
    res = run_bass_kernel_spmd(nc, in_maps, core_ids=list(range(NCORES)))
    out = np.stack(
        [res.results[i]["out"].reshape(B // NCORES, T, C) for i in range(NCORES)]
    )
    return out.reshape(B, T, C).astype(np.float32)



# revision 42
# speedup vs baseline: 1.0721x; 1.0721x over previous
"""Trainium2 Bass kernel for a dense transformer block (B=4096, T=32, C=180,
H=6 heads, head_dim=30): LN -> causal MHA -> residual -> LN -> MLP(180->720
->180, relu) -> residual.

Data-parallel over 8 NeuronCores: batch split 512 sequences/core, weights
replicated. Per 512-token tile:
  - LN1 with rstd = exp(-0.5*ln(var+eps)) so all scalar-engine activations
    stay inside one hw function table set (no ACT_TABLE_LOAD churn).  The x
    DMA and LN1 statistics for tile i+1 run during tile i's body, so each
    tile's front starts at the affine apply instead of the bn_stats chain.
  - QK projections in channel-major [ch, tok]; scores computed TRANSPOSED
    (stationary=k block, moving=q block) so softmax weights come out in
    [s, t] layout and feed the AV matmul directly as the stationary
    operand -- no PE transpose / PSUM round trip of the attention matrix.
    Tiny 32x32 score/AV matmuls are emitted rotating across PE row strips
    so LDWEIGHTS pulls ahead and the 16 subarrays run concurrently.
  - V computed token-major [tok, d] (stationary=h^T chunk, moving=wv), with
    a ones-column injected per head so the AV matmul also produces the
    softmax denominator; all 96 AV matmuls land in two PSUM tiles and the
    softmax normalize is ONE reciprocal + ONE multiply per chunk pair.
  - exp() writes bf16 straight from PSUM; causal mask applied in bf16 SBUF.
  - MLP hidden padded 720->6x128 and w2 output groups 90->128 so every MLP
    stationary is 128 columns (enables walrus fast-weight-load).
  - PE clock-gate (HAM) management: the PE idles at K=4/8 (1.2 GHz) and
    only reaches 2.4 GHz after a gap-free ~3.4us activity window, so a
    warmup burst of junk matmuls runs during the initial weight DMA and
    small junk-matmul "fillers" bridge the known dependency stalls (QK
    waiting on hT copies, scores/AV waiting on exp, out-transposes waiting
    on mlpT copies).  Without them the PE re-throttles every tile and all
    matmul streaming runs at half rate.
  - Software pipeline: tile i's MLP matmuls fill tile i+1's attention
    dependency windows; out-transposes run after LN2 so the mlpT copies
    complete under the LN2 transposes.

Self-contained: only needs numpy + the concourse (bass) package the runtime
environment provides on sys.path.
"""

import ml_dtypes
import numpy as np

import concourse.bass as bass
import concourse.tile as tile
from concourse import mybir
from concourse.bass_utils import run_bass_kernel_spmd

B, T, C, H, HD = 4096, 32, 180, 6, 30
DP = 32                   # padded head dim
D4 = 4 * C                # 720
D4P = 768                 # 720 padded to 6 groups of 128 (FWL wants 128-col
                          # stationaries; NumWeights==128 enables fast weight
                          # load in walrus codegen)
D4P = 768                 # 720 padded to 6 groups of 128 (FWL wants 128-col
                          # stationaries: NumWeights==128 enables fast weight
                          # load in walrus codegen)
EPS = 1e-5
NCORES = 8
F32 = mybir.dt.float32
MMDT = mybir.dt.bfloat16  # matmul operand dtype (PSUM accumulation stays fp32)

KA, KB = 128, C - 128     # contraction split of C=180
NT = 512                  # tokens per tile (16 sequences)
NCH = NT // 128           # 4 chunks of 128 tokens per tile
QKW = 2 * H * DP          # 384 columns of fused q|k weight
STAGE = 11                # debug: truncate tile body after stage N (11=full)


def _split_multi_waits(nc):
    """This walrus build accepts at most one sync wait per instruction; hoist
    extra waits emitted by Tile onto NoOps inserted just before, same engine."""
    ctr = 0
    for fn in nc.m.functions:
        for bb in fn.blocks:
            lst = bb.instructions
            i = 0
            while i < len(lst):
                inst = lst[i]
                si = inst.sync_info
                waits = list(si.on_wait) if si and si.on_wait else []
                if len(waits) > 1:
                    nops = []
                    for w in waits[:-1]:
                        ctr += 1
                        nop = mybir.InstNoOp(
                            name=f"T-wsplit-{ctr}",
                            opcode="NoOp",
                            engine=inst.engine,
                            ins=[],
                            outs=[],
                        )
                        nop.sync_info = mybir.SyncInfo(on_wait=[w], on_update=[])
                        nops.append(nop)
                    si.on_wait = waits[-1:]
                    inst.sync_info = si
                    for j, nop in enumerate(nops):
                        lst.insert(i + j, nop)
                    i += len(nops)
                i += 1


def _bc(ap, new_ap, extra_offset=0):
    """AP with a hand-built access-pattern list (stride-0 broadcasts etc.)."""
    return bass.AP(tensor=ap.tensor, offset=ap.offset + extra_offset, ap=new_ap)


def build(n_tok, has_bqkv, has_b1, has_b2, legalize=True):
    nc = bass.Bass()
    ts, ds = bass.ts, bass.ds
    AF = mybir.ActivationFunctionType
    ALU = mybir.AluOpType
    ntiles = n_tok // NT

    x_d = nc.declare_dram_parameter("x", [n_tok, C], F32, isOutput=False)
    wqk_d = nc.declare_dram_parameter("wqk", [C, QKW], MMDT, isOutput=False)
    wv_d = nc.declare_dram_parameter("wv", [C, H * DP], MMDT, isOutput=False)
    bqkv_d = nc.declare_dram_parameter("bqkv", [QKW, 1], F32, isOutput=False)
    bvrep_d = nc.declare_dram_parameter("bvrep", [128, H * DP], F32, isOutput=False)
    w1_d = nc.declare_dram_parameter("w1", [C, D4P], MMDT, isOutput=False)
    b1_d = nc.declare_dram_parameter("b1", [D4P, 1], F32, isOutput=False)
    w2t_d = nc.declare_dram_parameter("w2t", [128, 6, 256], MMDT, isOutput=False)
    b2_d = nc.declare_dram_parameter("b2", [C, 1], F32, isOutput=False)
    mask_d = nc.declare_dram_parameter("maskT", [128, T], MMDT, isOutput=False)
    id_d = nc.declare_dram_parameter("ident", [128, 128], F32, isOutput=False)
    out_d = nc.declare_dram_parameter("out", [n_tok, C], F32, isOutput=True)

    with tile.TileContext(nc) as tc:
        with (
            tc.tile_pool(name="const", bufs=1) as const,
            tc.tile_pool(name="tok", bufs=4) as tok,
            tc.tile_pool(name="ln", bufs=2) as lnp,
            tc.tile_pool(name="tr", bufs=2) as tr,
            tc.tile_pool(name="qk", bufs=2) as qkp,
            tc.tile_pool(name="at", bufs=2) as at,
            tc.tile_pool(name="mlp", bufs=2) as mlpp,
            tc.tile_pool(name="ps", bufs=1, space="PSUM") as ps,
        ):
            # ---- constants / weights ----
            ident = const.tile([128, 128], F32)
            nc.sync.dma_start(out=ident, in_=id_d[:, :])
            maskT = const.tile([128, T], MMDT)
            nc.sync.dma_start(out=maskT, in_=mask_d[:, :])

            wqkA = const.tile([KA, QKW], MMDT)
            nc.sync.dma_start(out=wqkA, in_=wqk_d[0:KA, :])
            wqkB = const.tile([KB, QKW], MMDT)
            nc.sync.dma_start(out=wqkB, in_=wqk_d[KA:C, :])
            wvA = const.tile([KA, H * DP], MMDT)
            nc.gpsimd.dma_start(out=wvA, in_=wv_d[0:KA, :])
            wvB = const.tile([KB, H * DP], MMDT)
            nc.gpsimd.dma_start(out=wvB, in_=wv_d[KA:C, :])
            w1A = const.tile([KA, D4P], MMDT)
            nc.gpsimd.dma_start(out=w1A, in_=w1_d[0:KA, :])
            w1B = const.tile([KB, D4P], MMDT)
            nc.gpsimd.dma_start(out=w1B, in_=w1_d[KA:C, :])
            w2t = const.tile([128, 6, 256], MMDT)
            nc.gpsimd.dma_start(out=w2t, in_=w2t_d[:, :, :])

            # q|k band biases: (offset, rows) for qA,qB,kA,kB
            BANDS = [(0, 128), (128, 64), (192, 128), (320, 64)]
            bqk = []
            for bi, (c0, rows) in enumerate(BANDS):
                t_ = const.tile([rows, 1], F32, tag=f"bqk{bi}")
                if has_bqkv:
                    nc.sync.dma_start(out=t_, in_=bqkv_d[c0 : c0 + rows, :])
                bqk.append(t_)
            bvrep = const.tile([128, H * DP], F32)
            if has_bqkv:
                nc.sync.dma_start(out=bvrep, in_=bvrep_d[:, :])
            b1t = const.tile([128, 6, 1], F32)
            if has_b1:
                nc.sync.dma_start(
                    out=b1t, in_=b1_d.rearrange("(g p) one -> p g one", g=6)
                )
            b2t = const.tile([90, 2, 1], F32)
            if has_b2:
                nc.sync.dma_start(
                    out=b2t, in_=b2_d.rearrange("(m p) one -> p m one", m=2)
                )
            epst = const.tile([128, 1], F32)
            nc.vector.memset(epst, EPS)
            ones1 = const.tile([128, 1], F32)
            nc.vector.memset(ones1, 1.0)
            identb = const.tile([128, 128], MMDT)
            nc.vector.tensor_copy(identb, ident)

            # ---- HAM warmup ----
            # The PE clock gate defaults to K=4/8 (1.2 GHz) and only reaches
            # 2.4 GHz after a fully-busy 4096-cycle activity window.  Burn
            # ~7us of dummy matmuls on memset data during the initial
            # weight-DMA wait so real matmuls start at full rate.
            wu_s = const.tile([128, 128], MMDT)
            nc.vector.memset(wu_s, 0.25)
            wu_m = const.tile([128, 512], MMDT)
            nc.vector.memset(wu_m, 0.25)
            wu_ps = [
                ps.tile([128, 512], F32, tag=t, name=f"wu{t}")
                for t in ("P4", "P5")
            ]
            for wi in range(24):
                nc.tensor.matmul(
                    wu_ps[wi % 2], wu_s, wu_m, start=True, stop=True
                )

            fill_ctr = [0]

            def fill(n, fp=None):
                """Bridge a known PE stall with junk matmuls so the HAM
                activity monitor keeps the clock gate at K=8/8 (any ~us idle
                re-throttles the PE to 1.2 GHz).  Writes go to the P3 slot,
                whose last real reader is two iterations old at every call
                site; reusing one tile per iteration keeps the repeat writes
                WAW-ordered on the PE queue with no semaphore waits."""
                if fp is None:
                    fill_ctr[0] += 1
                    fp = ps.tile([64, 64], F32, tag="P3", name=f"fl{fill_ctr[0]}")
                for _ in range(n):
                    nc.tensor.matmul(
                        fp, wu_s[:, 0:64], wu_m[:, 0:64], start=True, stop=True
                    )
                return fp

            # ---- HAM warmup ----
            # The PE clock gate defaults to K=4/8 (1.2 GHz) and only reaches
            # 2.4 GHz after a fully-busy 4096-cycle activity window.  The
            # baseline ran cold for 98% of the kernel (HAM fired at t=977us).
            # Burn ~7us of dummy matmuls on memset data during the initial
            # weight-DMA wait so every real matmul streams at full rate.
            wu_s = const.tile([128, 128], MMDT)
            nc.vector.memset(wu_s, 0.25)
            wu_m = const.tile([128, 512], MMDT)
            nc.vector.memset(wu_m, 0.25)
            wu_ps = [
                ps.tile([128, 512], F32, tag=t, name=f"wu{t}")
                for t in ("PT", "M0")
            ]
            for wi in range(24):
                nc.tensor.matmul(
                    wu_ps[wi % 2], wu_s, wu_m, start=True, stop=True
                )

            fill_ctr = [0]

            def fill(n, fp=None):
                """Bridge a known PE stall with junk matmuls so the HAM
                activity monitor keeps the clock gate at K=8/8 (any ~us idle
                re-throttles the PE to 1.2 GHz).  Writes go to the P3 slot,
                whose last real reader is two iterations old at every call
                site; reusing one tile per iteration keeps the repeat writes
                WAW-ordered on the PE queue with no semaphore waits."""
                if fp is None:
                    fill_ctr[0] += 1
                    fp = ps.tile([64, 64], F32, tag="P3", name=f"fl{fill_ctr[0]}")
                for _ in range(n):
                    nc.tensor.matmul(
                        fp, wu_s[:, 0:64], wu_m[:, 0:64], start=True, stop=True
                    )
                return fp

            def ln_t(src, sfx):
                """LN of [128, NCH, C] (rstd via exp(-ln/2)) fused with the
                per-chunk PE transposes: each chunk's transpose is emitted
                right after its affine apply (applies split DVE/Act) so the
                PE starts as early as possible. Returns hTA [128,512] +
                hTB [52,512] (bf16, channel-major)."""
                mv = lnp.tile([128, NCH, nc.vector.BN_AGGR_DIM], F32, tag=f"mv{sfx}")
                for j in range(NCH):
                    st = lnp.tile([128, nc.vector.BN_STATS_DIM], F32, tag=f"st{sfx}")
                    nc.vector.bn_stats(out=st, in_=src[:, j, :])
                    nc.vector.bn_aggr(out=mv[:, j, :], in_=st)
                lnv = lnp.tile([128, NCH], F32, tag=f"lnv{sfx}")
                nc.scalar.activation(
                    out=lnv, in_=mv[:, :, 1], func=AF.Ln, bias=epst, scale=1.0
                )
                rstd = lnp.tile([128, NCH], F32, tag=f"rstd{sfx}")
                nc.scalar.activation(out=rstd, in_=lnv, func=AF.Exp, scale=-0.5)
                nmr = lnp.tile([128, NCH], F32, tag=f"nmr{sfx}")
                nc.vector.scalar_tensor_tensor(
                    out=nmr, in0=mv[:, :, 0], scalar=-1.0, in1=rstd,
                    op0=ALU.mult, op1=ALU.mult,
                )
                xh = lnp.tile([128, NCH, C], MMDT, tag=f"xh{sfx}")
                pT = ps.tile([128, 1024], MMDT, tag="PT", name=f"pT{sfx}")
                # A-half transposes first so the hTA copy overlaps the B-half
                # transposes on the PE (QK's first matmul only needs hTA).
                for j in range(NCH):
                    if j % 2 == 0:
                        nc.vector.scalar_tensor_tensor(
                            out=xh[:, j, :],
                            in0=src[:, j, :],
                            scalar=rstd[:, j : j + 1],
                            in1=_bc(nmr, [nmr.ap[0], [0, C]], j),
                            op0=ALU.mult,
                            op1=ALU.add,
                        )
                    else:
                        nc.scalar.activation(
                            out=xh[:, j, :],
                            in_=src[:, j, :],
                            func=AF.Identity,
                            bias=nmr[:, j : j + 1],
                            scale=rstd[:, j : j + 1],
                        )
                    nc.tensor.transpose(pTA[:, ts(j, 128)], xh[:, j, 0:128], identb)
                hTA = tr.tile([KA, 512], MMDT, tag=f"hTA{sfx}")
                nc.vector.tensor_copy(hTA, pTA)
                for j in range(NCH):
                    nc.tensor.transpose(
                        pTB[:, ts(j, 128)], xh[:, j, 128:C], identb,
                    )
                hTB = tr.tile([KB, 512], MMDT, tag=f"hTB{sfx}")
                nc.scalar.copy(hTB, pTB)
                return hTA, hTB

            def stage_front1(it):
                """LN1 -> T1 -> QK -> V -> scores for tile it."""
                st = {}
                tok0 = it * NT
                x_t = tok.tile([128, NCH, C], F32, tag="x_t")
                nc.sync.dma_start(
                    out=x_t,
                    in_=x_d[tok0 : tok0 + NT, :].rearrange("(n p) d -> p n d", p=128),
                )
                st["x_t"] = x_t

                hTA, hTB = ln_t(x_t, "1")
                fill(11)

                qk_ps = (
                    ps.tile([128, 512], F32, tag="Q0", name="qk0"),
                    ps.tile([64, 512], F32, tag="Q1", name="qk1"),
                    ps.tile([128, 512], F32, tag="Q2", name="qk2"),
                    ps.tile([64, 512], F32, tag="Q3", name="qk3"),
                )
                qk_sb = []
                for bi, (c0, rows) in enumerate(BANDS):
                    p_mm = qk_ps[bi]
                    nc.tensor.matmul(
                        p_mm, wqkA[:, c0 : c0 + rows], hTA, start=True, stop=False
                    )
                    nc.tensor.matmul(
                        p_mm, wqkB[:, c0 : c0 + rows], hTB, start=False, stop=True
                    )
                    dst = qkp.tile([rows, 512], MMDT, tag=f"qk{bi}")
                    eng_dve = bi % 2 == 0
                    if has_bqkv:
                        if eng_dve:
                            nc.vector.tensor_scalar(
                                out=dst, in0=p_mm, scalar1=bqk[bi], scalar2=None,
                                op0=ALU.add,
                            )
                        else:
                            nc.scalar.activation(
                                out=dst, in_=p_mm, func=AF.Identity,
                                bias=bqk[bi], scale=1.0,
                            )
                    elif eng_dve:
                        nc.vector.tensor_copy(dst, p_mm)
                    else:
                        nc.scalar.copy(dst, p_mm)
                    qk_sb.append(dst)
                qA, qB, kA, kB = qk_sb

                vps = [
                    ps.tile([128, 2, 256], F32, tag="Q2", name="vps01"),
                    ps.tile([128, 2, 256], F32, tag="Q3", name="vps23"),
                ]
                for j in range(NCH):
                    dst = vps[j // 2][:, j % 2, 0 : H * DP]
                    nc.tensor.matmul(
                        dst, hTA[:, ts(j, 128)], wvA, start=True, stop=False
                    )
                    nc.tensor.matmul(
                        dst, hTB[:, ts(j, 128)], wvB, start=False, stop=True
                    )
                vp = qkp.tile([128, NCH, H * DP], MMDT, tag="vp")
                nc.scalar.copy(vp[:, 0:2, :], vps[0][:, :, 0 : H * DP])
                nc.scalar.copy(vp[:, 2:4, :], vps[1][:, :, 0 : H * DP])
                nc.vector.tensor_copy(
                    _bc(vp, [vp.ap[0], [H * DP, NCH], [DP, H]], 30),
                    _bc(ones1, [ones1.ap[0], [0, NCH], [0, H]]),
                )
                st["vp"] = vp

                sc01 = ps.tile([128, 512], F32, tag="Q0", name="sc01")
                sc23 = ps.tile([128, 256], F32, tag="Q1", name="sc23")
                sc = [sc01[:, 0:256], sc01[:, 256:512],
                      sc23[:, 0:128], sc23[:, 128:256]]
                # Emission order rotates the PE row strip (= bank rr) between
                # consecutive matmuls so LDWEIGHTS pulls ahead of in-flight
                # MATMULs (pull-ahead requires a different row_grp) and the
                # 32x32 subarrays run concurrently.
                # banks 0/1 first so exp+mask of those banks overlap the
                # bank-2/3 matmuls; within a set, rotate row strips so
                # LDWEIGHTS pulls ahead.
                BANKSETS = ([(0, 0), (1, 1), (0, 4), (1, 5)], [(2, 2), (3, 3)])
                for bankset in BANKSETS:
                    for g in range(NCH):
                        for c in range(4):
                            sq = 4 * g + c
                            for rr, h in bankset:
                                col0 = 128 if h >= 4 else 0
                                qs, ks = (qA, kA) if h < 4 else (qB, kB)
                                pb = 32 * rr
                                nc.tensor.matmul(
                                    sc[rr][ds(32 * c, 32), ds(col0 + 32 * g, 32)],
                                    ks[ds(pb, 32), ts(sq, T)],
                                    qs[ds(pb, 32), ts(sq, T)],
                                    start=True,
                                    stop=True,
                                    tile_position=(pb, 32 * c),
                                )
                st["sc"] = sc
                return st

            def stage_exp(st):
                """exp + causal mask per head-row bank -> eT (bf16)."""
                sc = st["sc"]
                eT = at.tile([128, H * NCH, T], MMDT, tag="eT")
                for rr in range(4):
                    if rr < 2:
                        o_ap = _bc(
                            eT,
                            [eT.ap[0], [4 * NCH * T, 2], [T, NCH], [1, T]],
                            rr * NCH * T,
                        )
                        i_ap = _bc(
                            sc[rr], [sc[rr].ap[0], [128, 2], [T, NCH], [1, T]]
                        )
                        m_ap = _bc(
                            maskT, [maskT.ap[0], [0, 2], [0, NCH], [1, T]]
                        )
                    else:
                        o_ap = _bc(
                            eT, [eT.ap[0], [T, NCH], [1, T]], rr * NCH * T
                        )
                        i_ap = _bc(sc[rr], [sc[rr].ap[0], [T, NCH], [1, T]])
                        m_ap = _bc(maskT, [maskT.ap[0], [0, NCH], [1, T]])
                    nc.scalar.activation(out=o_ap, in_=i_ap, func=AF.Exp)
                    nc.vector.tensor_mul(o_ap, o_ap, m_ap)  # BISECT-A
                st["eT"] = eT

            def stage_av(st):
                """All AV matmuls into one 2-bank PSUM tile (denominators ride
                col 30 of each head slot), then one reciprocal + one scale."""
                eT, vp = st["eT"], st["vp"]
                avp = [
                    ps.tile([128, 2, 256], F32, tag="Q0", name="av01"),
                    ps.tile([128, 2, 256], F32, tag="Q1", name="av23"),
                ]
                # c fastest: consecutive MMs land on different row strips ->
                # LDWEIGHTS pull-ahead + subarray concurrency
                for h in range(H):
                    for g in range(NCH):
                        idx2 = h * NCH + g
                        for c in range(4):
                            nc.tensor.matmul(
                                avp[g // 2][ds(32 * c, 32), g % 2, ds(DP * h, 32)],
                                eT[ds(32 * c, 32), idx2, :],
                                vp[ds(32 * c, 32), g, ds(DP * h, 32)],
                                start=True,
                                stop=True,
                                tile_position=(32 * c, 32 * c),
                            )
                avs = at.tile([128, NCH, H, HD], F32, tag="avs", name="avs")
                rcp = at.tile([128, 2, 2, H], F32, tag="rcp")
                for gh in range(2):
                    nc.vector.reciprocal(
                        rcp[:, gh],
                        _bc(avp[gh], [avp[gh].ap[0], [256, 2], [DP, H]], 30),
                    )
                    nc.vector.tensor_mul(
                        avs[:, 2 * gh : 2 * gh + 2],
                        _bc(avp[gh],
                            [avp[gh].ap[0], [256, 2], [DP, H], [1, HD]]),
                        _bc(rcp, [rcp.ap[0], [H, 2], [1, H], [0, HD]],
                            gh * 2 * H),
                    )
                st["avs"] = avs

            def stage_resid_ln2(st):
                """x2 = x + attn; LN2; T2 transposes."""
                x_t, avs = st["x_t"], st["avs"]
                x2 = tok.tile([128, NCH, C], F32, tag="x2")
                nc.vector.tensor_add(  # BISECT-A
                    _bc(x2, [x2.ap[0], [C, NCH], [HD, H], [1, HD]]),
                    _bc(x_t, [x_t.ap[0], [C, NCH], [HD, H], [1, HD]]),
                    avs,
                )
                if has_bqkv:
                    x2v = _bc(x2, [x2.ap[0], [C, NCH], [HD, H], [1, HD]])
                    nc.vector.tensor_add(
                        x2v,
                        x2v,
                        _bc(bvrep, [bvrep.ap[0], [0, NCH], [DP, H], [1, HD]]),
                    )
                st["x2"] = x2
                fill(6)
                st["h2"] = ln_t(x2, "2")

            def stage_mlp1(st, gms):
                h2A, h2B = st["h2"]
                M1ROT = ["M0", "M1", "M2"]
                m1 = st.setdefault("m1", {})
                for gm in gms:
                    p_mm = ps.tile([128, 512], F32, tag=M1ROT[gm % 3])
                    nc.tensor.matmul(
                        p_mm, w1A[:, ds(128 * gm, 128)], h2A, start=True, stop=False
                    )
                    nc.tensor.matmul(
                        p_mm, w1B[:, ds(128 * gm, 128)], h2B, start=False, stop=True
                    )
                    dst = mlpp.tile([128, 512], MMDT, tag=f"m1_{gm}")
                    if gm != 0:
                        nc.scalar.activation(
                            out=dst,
                            in_=p_mm,
                            func=AF.Relu,
                            bias=b1t[:, gm, :] if has_b1 else 0.0,
                            scale=1.0,
                        )
                    elif has_b1:
                        nc.vector.tensor_scalar(
                            out=dst,
                            in0=p_mm,
                            scalar1=b1t[:, gm, :],
                            scalar2=0.0,
                            op0=ALU.add,
                            op1=ALU.max,
                        )
                    else:
                        nc.vector.tensor_scalar(
                            out=dst,
                            in0=p_mm,
                            scalar1=0.0,
                            scalar2=None,
                            op0=ALU.max,
                        )
                    m1[gm] = dst

            def stage_mlp2(st):
                m1 = st["m1"]
                mlpT = []
                for mm in range(2):
                    p_mm = ps.tile([128, 512], F32, tag=("M1", "M2")[mm])
                    for k in range(6):
                        nc.tensor.matmul(
                            p_mm,
                            w2t[:, k, ds(128 * mm, 128)],
                            m1[k],
                            start=(k == 0),
                            stop=(k == 5),
                        )
                    dst = mlpp.tile([90, 512], MMDT, tag=f"mlpT{mm}")
                    p_top = p_mm[0:90, :]
                    if has_b2:
                        if mm == 0:
                            nc.vector.tensor_scalar(
                                out=dst, in0=p_top, scalar1=b2t[:, mm, :],
                                scalar2=None, op0=ALU.add,
                            )
                        else:
                            nc.scalar.activation(
                                out=dst, in_=p_top, func=AF.Identity,
                                bias=b2t[:, mm, :], scale=1.0,
                            )
                    elif mm == 0:
                        nc.vector.tensor_copy(dst, p_top)
                    else:
                        nc.scalar.copy(dst, p_top)
                    mlpT.append(dst)
                st["mlpT"] = mlpT

            def stage_out(st, it):
                """Transpose back + final residual + store for tile it."""
                tok0 = it * NT
                x2, mlpT = st["x2"], st["mlpT"]
                of = tok.tile([128, NCH, C], F32, tag="of")
                po = ps.tile([128, 2, NCH, 90], MMDT, tag="PT", name="po")
                for mm in range(2):
                    for j in range(NCH):
                        nc.tensor.transpose(
                            po[:, mm, j, :], mlpT[mm][:, ts(j, 128)],
                            identb[0:90, 0:90],
                        )
                for mm in range(2):
                    o_ap = _bc(of, [of.ap[0], [C, NCH], [1, 90]], 90 * mm)
                    i_ap = _bc(x2, [x2.ap[0], [C, NCH], [1, 90]], 90 * mm)
                    nc.vector.tensor_add(o_ap, i_ap, po[:, mm, :, :])
                nc.sync.dma_start(
                    out=out_d[tok0 : tok0 + NT, :].rearrange(
                        "(n p) d -> p n d", p=128
                    ),
                    in_=of,
                )

            # Software-pipelined schedule: tile i's MLP/out (back half) is
            # emitted inside tile i+1's attention stalls, so every engine
            # stream keeps cross-tile work adjacent to the other tiles'
            # latency chains.
            prev = None
            prev_it = -1
            for it in range(ntiles):
                st = stage_front1(it)
                stage_exp(st)
                if prev is not None:
                    stage_mlp1(prev, range(0, 1))
                stage_av(st)
                if prev is not None:
                    stage_mlp1(prev, range(1, 3))
                    stage_mlp2(prev)
                    stage_out(prev, prev_it)
                stage_resid_ln2(st)
                stage_mlp1(st, range(3, 6))
                prev, prev_it = st, it
            stage_mlp1(prev, range(0, 3))
            stage_mlp2(prev)
            stage_out(prev, prev_it)

    if legalize:
        _split_multi_waits(nc)
    return nc


def _prep(wq, wk, wv, g1, be1, g2, be2, w1, b1, w2, b2):
    f = np.float32
    wq, wk, wv = (np.asarray(a, f) for a in (wq, wk, wv))
    g1, be1 = np.asarray(g1, f), np.asarray(be1, f)
    g2, be2 = np.asarray(g2, f), np.asarray(be2, f)
    wqk = np.zeros((C, QKW), f)
    wvm = np.zeros((C, H * DP), f)
    for h in range(H):
        wqk[:, DP * h : DP * h + HD] = wq[h] * f(HD) ** -0.5
        wqk[:, H * DP + DP * h : H * DP + DP * h + HD] = wk[h]
        wvm[:, DP * h : DP * h + HD] = wv[h]
    bqk = (be1 @ wqk).reshape(QKW, 1)
    bv = be1 @ wvm
    bvrep = np.tile(bv[None, :], (128, 1))
    wqk *= g1[:, None]
    wvm *= g1[:, None]
    w1 = np.asarray(w1, f)
    w1e = g2[:, None] * w1
    b1e = np.asarray(b1, f) + be2 @ w1
    # pad the 720-dim hidden to 6 groups of 128 (cols 120..127 of each group
    # are zero) so every MLP stationary is 128 columns wide -> FWL kicks in
    w1p = np.zeros((C, D4P), f)
    b1p = np.zeros((D4P, 1), f)
    for gm in range(6):
        w1p[:, 128 * gm : 128 * gm + 120] = w1e[:, 120 * gm : 120 * gm + 120]
        b1p[128 * gm : 128 * gm + 120, 0] = b1e[120 * gm : 120 * gm + 120]
    w2 = np.asarray(w2, f)
    w2tp = np.zeros((128, 6, 256), f)
    for k in range(6):
        for mm in range(2):
            w2tp[0:120, k, 128 * mm : 128 * mm + 90] = w2[
                120 * k : 120 * k + 120, 90 * mm : 90 * mm + 90
            ]
    maskT = np.tile(
        (np.arange(T)[:, None] <= np.arange(T)[None, :]).astype(f), (128 // T, 1)
    )
    ident = np.eye(128, dtype=f)
    bf = ml_dtypes.bfloat16
    return {
        "wqk": np.ascontiguousarray(wqk.astype(bf)),
        "wv": np.ascontiguousarray(wvm.astype(bf)),
        "bqkv": np.ascontiguousarray(bqk),
        "bvrep": np.ascontiguousarray(bvrep),
        "w1": np.ascontiguousarray(w1p.astype(bf)),
        "b1": np.ascontiguousarray(b1p),
        "w2t": np.ascontiguousarray(w2tp.astype(bf)),
        "b2": np.ascontiguousarray(np.asarray(b2, f).reshape(C, 1)),
        "maskT": np.ascontiguousarray(maskT.astype(bf)),
        "ident": ident,
    }


def kernel(x, wq, wk, wv, g1, be1, g2, be2, w1, b1, w2, b2):
    x = np.asarray(x, np.float32)
    shared = _prep(wq, wk, wv, g1, be1, g2, be2, w1, b1, w2, b2)
    n_tok = (B // NCORES) * T
    nc = build(
        n_tok,
        bool(np.any(shared["bqkv"])) or bool(np.any(shared["bvrep"])),
        bool(np.any(shared["b1"])),
        bool(np.any(shared["b2"])),
    )
    shards = x.reshape(NCORES, n_tok, C)
    in_maps = [
        {"x": np.ascontiguousarray(shards[i]), **shared} for i in range(NCORES)
    ]
    res = run_bass_kernel_spmd(nc, in_maps, core_ids=list(range(NCORES)))
    out = np.stack(
        [res.results[i]["out"].reshape(B // NCORES, T, C) for i in range(NCORES)]
    )
    return out.reshape(B, T, C).astype(np.float32)

